# revision 58
# baseline (speedup 1.0000x reference)
"""Trainium2 Bass kernel for nn_BertIntermediate (QuantizeLinear + exact GELU).

Reference computation:
    xq = fake_quant(x)   # symmetric per-tensor int8 fake quant, scale = max|x|/127
    Wq = fake_quant(W)
    h  = xq @ Wq.T + b
    out = h * 0.5 * (1 + erf(h/sqrt(2)))

Numerical scheme — e4m3 split + DoubleRow fp8 matmul:
  v = x/scale; the reference rounds q = rne(v), an integer in [-128, 127].
  Split q = q8 + r with q8 = e4m3(v) (a single tensor_scalar whose fp8 SBUF
  write performs the rounding) and r = rne(q - q8), computed exactly on
  device via the fp32 magic-number trick from t = v + 1.5*2^23 (which
  materializes rne(v)) and the actually-written q8. r is in [-4, 4] and
  exactly representable in fp8e4. Then
      qx.qw = qx8.qw8 + qx8.rw + rx.qw8 + rx.rw
  and the rx.rw term is dropped (~4e-4 of output absmax). The three kept
  groups run as fp8e4 DoubleRow matmuls (two contraction rows per PE pass,
  0.5 cycles/row): 6N cycles per [128, N] output tile vs bf16's 8N, i.e.
  PE time 82us vs 109us per core. Measured end-to-end rel err 8.2e-3
  (threshold 2e-2).

  The scales factor out of the GEMM into the scalar-engine activation:
  out = gelu(sx*sw*psum + bias), evacuated directly to bf16 (halves the
  output DMA; bf16 adds ~2e-3 rel max, still way in budget).

Sharding (8 cores): 2D grid, 4-way over tokens x 2-way over intermediate
(min DMA: ~10.5 MB in + 8.4 MB out per core). The global quantization
scales need max|x|, max|W| over the FULL tensors: each core reduces a
distinct 1/8 shard (host permutes token/I columns so its shard is block 0
of its inputs), a PE-transpose folds per-partition maxes, and one tiny
padded AllGather + local max yields the global scales on-device. The
collective has a fixed ~15us cost in the perf model and its input DMA
queues behind all previously-issued transfers, so every bulk DMA after the
shards is sync-gated on the cc ship to keep the queue clear.

Schedule: the AllGather window streams the remaining W/x chunks; after the
scales land, single-pass direct casts unblock the PE within ~3us while the
residual chains (t, d=t-q8, r8=d-magic) trail on ACT/DVE/Pool. Matmul
groups are emitted kp-outer across each I-quad's four PSUM pairs with the
rx-dependent third term per-group last, closing groups progressively so
gelu evacuations overlap the remaining matmuls.
"""

import numpy as np

import concourse.bass as bass
import concourse.mybir as mybir
from concourse import bass_utils
from concourse.tile import TileContext
from concourse.tile_rust import add_dep_helper

F32 = mybir.dt.float32
BF16 = mybir.dt.bfloat16
FP8 = mybir.dt.float8e4
MAGIC = 12582912.0  # 1.5 * 2**23: fp32 add/sub rounds to nearest int (RNE)
N_CORES = 8
TI, II = 4, 2  # token-quarters x intermediate-halves

# Full problem dims
B, S, H, I = 16, 512, 1024, 4096
M = B * S  # 8192 tokens

DR = mybir.MatmulPerfMode.DoubleRow


def _split_sync_waits(nc, max_waits=1):
    """Walrus in this container rejects instructions carrying more than a
    couple of sync-wait commands ("Too many sync wait commands"). Hoist excess
    waits onto single-wait nops inserted just before the instruction on the
    same engine queue — sequencers process in order, so semantics are
    unchanged."""
    n = 0
    for fn in nc.m.functions:
        for blk in fn.blocks:
            new_insts = []
            for inst in blk.instructions:
                si = inst.sync_info
                waits = list(si.on_wait or []) if si is not None else []
                if len(waits) > max_waits:
                    keep = waits[-max_waits:]
                    for w in waits[:-max_waits]:
                        n += 1
                        nop = mybir.InstNoOp(
                            name=f"I-waitsplit-{n}",
                            ins=[],
                            outs=[],
                            engine=inst.engine,
                        )
                        nop.sync_info = mybir.SyncInfo(on_wait=[w], on_update=[])
                        new_insts.append(nop)
                    inst.sync_info = mybir.SyncInfo(
                        on_wait=keep, on_update=list(si.on_update or [])
                    )
                new_insts.append(inst)
            blk.instructions = new_insts


def build(h=H, m_core=M // TI, i_core=I // II):
    """Build the SPMD Bass program for one core's block."""
    kt = h // 128          # contraction k-tiles
    n_it = i_core // 128   # output I-tiles
    n_tg = m_core // 512   # token groups
    xsh_cols = m_core // II   # this core's distinct x max-shard (cols 0..)
    wsh_cols = i_core // TI   # this core's distinct W max-shard (cols 0..)

    nc = bass.Bass(num_devices=N_CORES)
    xT = nc.dram_tensor("xT", [h, m_core], F32, kind="ExternalInput")
    wT = nc.dram_tensor("wT", [h, i_core], F32, kind="ExternalInput")
    bias = nc.dram_tensor("bias", [128, n_it], F32, kind="ExternalInput")
    outT = nc.dram_tensor("outT", [i_core, m_core], BF16, kind="ExternalOutput")
    CCW = 16  # payload padded to 64 B/rank; only the first 2 floats are used
    cc_w = nc.dram_tensor("cc_w", [1, CCW], F32, kind="Internal")
    cc_wo = nc.dram_tensor("cc_wo", [1, CCW * N_CORES], F32, kind="Internal",
                           addr_space="Shared")
    cc_x = nc.dram_tensor("cc_x", [1, CCW], F32, kind="Internal")
    cc_xo = nc.dram_tensor("cc_xo", [1, CCW * N_CORES], F32, kind="Internal",
                           addr_space="Shared")
    ident = nc.inline_tensor(np.eye(128, dtype=np.float32), name="ident128")

    groups = [list(range(N_CORES))]

    with TileContext(nc) as tc:
        with (
            tc.tile_pool(name="res", bufs=1) as res,
            tc.tile_pool(name="work", bufs=4) as work,
            tc.tile_pool(name="small", bufs=1) as small,
            tc.tile_pool(name="psum", bufs=4, space="PSUM") as pp,
            tc.tile_pool(name="evac", bufs=4) as evac,
        ):
            # persistent fp8 split tensors, layout [128, (k, cols)]
            qx8 = res.tile([128, kt * m_core], FP8, tag="qx8")
            rx8 = res.tile([128, kt * m_core], FP8, tag="rx8")
            qw8 = res.tile([128, kt * i_core], FP8, tag="qw8")
            rw8 = res.tile([128, kt * i_core], FP8, tag="rw8")
            # x max-shard staged in f32 until the scales arrive
            xsh = res.tile([128, kt * xsh_cols], F32, tag="xsh")
            wf0 = res.tile([128, kt * wsh_cols], F32, tag="wf0")

            macc = small.tile([128, 2 * kt + 2], F32, tag="macc")
            idt = small.tile([128, 128], F32, tag="idt")
            nc.sync.dma_start(idt[:], ident[:, :])

            # ---- phase 0: shard staging + maxes; W scales collective first
            # so W q0/q1 quantize during x's collective window ----
            for k in range(kt):
                nc.sync.dma_start(
                    wf0[:, k * wsh_cols:(k + 1) * wsh_cols],
                    wT[k * 128:(k + 1) * 128, 0:wsh_cols],
                )
                nc.vector.tensor_reduce(
                    macc[:, kt + 1 + k:kt + 2 + k],
                    wf0[:, k * wsh_cols:(k + 1) * wsh_cols],
                    axis=mybir.AxisListType.X,
                    op=mybir.AluOpType.max, apply_absolute_value=True,
                )
            for k in range(kt):
                if k < kt - 1:
                    parts = [(0, xsh_cols)]
                else:
                    parts = [(0, xsh_cols - 256), (xsh_cols - 256, xsh_cols)]
                for pi, (c0, c1) in enumerate(parts):
                    nc.sync.dma_start(
                        xsh[:, k * xsh_cols + c0:k * xsh_cols + c1],
                        xT[k * 128:(k + 1) * 128, c0:c1],
                    )
                    nc.vector.tensor_reduce(
                        macc[:, k:k + 1] if pi == 0 else macc[:, kt:kt + 1],
                        xsh[:, k * xsh_cols + c0:k * xsh_cols + c1],
                        axis=mybir.AxisListType.X,
                        op=mybir.AluOpType.max, apply_absolute_value=True,
                    )
            czero = small.tile([1, CCW], F32, tag="czero")
            nc.vector.memset(czero[:], 0.0)
            nc.sync.dma_start(cc_w[0:1, 2:CCW], czero[0:1, 2:CCW])
            nc.sync.dma_start(cc_x[0:1, 2:CCW], czero[0:1, 2:CCW])
            bt = small.tile([128, n_it], F32, tag="bt")
            nc.sync.dma_start(bt[:], bias[:, :])

            gm2 = small.tile([128, 2], F32, tag="gm2")
            lmax = small.tile([1, 2], F32, tag="lmax")
            sxsw = small.tile([128, 2], F32, tag="sxsw")
            inv = small.tile([128, 2], F32, tag="inv")
            g6t = small.tile([128, CCW * N_CORES], F32, tag="g6t")
            g6x = small.tile([128, CCW * N_CORES], F32, tag="g6x")
            gmxw = small.tile([128, CCW], F32, tag="gmxw")
            gmxx = small.tile([128, CCW], F32, tag="gmxx")
            n_wq = i_core // wsh_cols  # 4 I-quads

            # ---- quantize helpers ----
            def t_pass(t_eng, dst, src_ap, invc):
                if t_eng is nc.scalar:
                    nc.scalar.activation(
                        dst, src_ap, mybir.ActivationFunctionType.Copy,
                        bias=MAGIC, scale=invc,
                    )
                else:
                    t_eng.tensor_scalar(dst, src_ap, invc, MAGIC,
                                        op0=mybir.AluOpType.mult,
                                        op1=mybir.AluOpType.add)

            def m_pass(eng, dst, src_ap):
                """dst = src - MAGIC (fp8 write rounds to e4m3)."""
                if eng is nc.scalar:
                    nc.scalar.activation(
                        dst, src_ap, mybir.ActivationFunctionType.Copy,
                        bias=-MAGIC, scale=1.0,
                    )
                else:
                    eng.tensor_scalar(dst, src_ap, MAGIC, None,
                                      op0=mybir.AluOpType.subtract)

            # ---- scales: one AllGather carries both maxes ----
            for c in (1, 0):
                lo, hi = (kt + 1, 2 * kt + 1) if c == 1 else (0, kt + 1)
                nc.vector.tensor_reduce(
                    gm2[:, c:c + 1], macc[:, lo:hi], axis=mybir.AxisListType.X,
                    op=mybir.AluOpType.max,
                )
                gmt = pp.tile([1, 128], F32, tag="ps", name=f"gmt{c}")
                nc.tensor.transpose(gmt[:], gm2[:, c:c + 1], idt[:])
                nc.vector.tensor_reduce(
                    lmax[:, c:c + 1], gmt[:], axis=mybir.AxisListType.X,
                    op=mybir.AluOpType.max,
                )
            cc_dma = nc.sync.dma_start(cc_x[0:1, 0:2], lmax[0:1, 0:2])
            nc.gpsimd.collective_compute(
                "AllGather", mybir.AluOpType.bypass, replica_groups=groups,
                ins=[cc_x[:, :]], outs=[cc_xo[:, :]],
            )
            g6_dma = nc.sync.dma_start(
                g6x[:], cc_xo[0:1, :].broadcast_to([128, CCW * N_CORES])
            )
            nc.vector.tensor_reduce(
                gmxx[:], g6x[:, :].rearrange("p (r s) -> p s r", r=N_CORES),
                axis=mybir.AxisListType.X, op=mybir.AluOpType.max,
            )
            nc.vector.tensor_scalar_mul(sxsw[:], gmxx[:, 0:2], 1.0 / 127.0)
            nc.vector.reciprocal(inv[:], sxsw[:])
            ss = small.tile([128, 1], F32, tag="ss")
            nc.vector.tensor_tensor(
                ss[:], sxsw[:, 0:1], sxsw[:, 1:2], op=mybir.AluOpType.mult
            )

            # rest-of-input DMAs stream during/after the collective window;
            # gated on the cc ship so they cannot block it in the DMA FIFO.
            # All W quads first (their direct casts are PE gates), x rest after.
            wrest = {}
            for q in (1, 2, 3):
                for k in range(kt):
                    wf = work.tile([128, wsh_cols], F32, tag="wf", bufs=14,
                                   name=f"wf_{q}_{k}")
                    d = nc.sync.dma_start(
                        wf[:],
                        wT[k * 128:(k + 1) * 128,
                           q * wsh_cols:(q + 1) * wsh_cols],
                    )
                    add_dep_helper(d.ins, cc_dma.ins, sync=True,
                                   reason="keep DMA clear for cc ship")
                    wrest[(q, k)] = wf
            xrest = {}
            rw = m_core - xsh_cols
            for k in range(kt):
                xf = work.tile([128, rw], F32, tag="xf", bufs=4,
                               name=f"xrest_{k}")
                for h in range(2):
                    d = nc.sync.dma_start(
                        xf[:, h * 512:(h + 1) * 512],
                        xT[k * 128:(k + 1) * 128,
                           xsh_cols + h * 512:xsh_cols + (h + 1) * 512],
                    )
                    add_dep_helper(d.ins, cc_dma.ins, sync=True,
                                   reason="keep DMA clear for cc ship")
                xrest[k] = xf

            # ---- direct-cast front: qx8/qw8 are single tensor_scalar
            # casts (the fp8 write does the e4m3 rounding), so the PE's
            # qw.qx parts are gated only by these; the magic-rounded
            # residual chains (t, d, r8) trail behind ----
            def cast8(eng, dst, src_ap, invc):
                eng.tensor_scalar(dst, src_ap, invc, None,
                                  op0=mybir.AluOpType.mult)

            # layer 1: direct casts. DVE leads with the k0/k1 pairs the
            # first matmul needs; Pool covers the remaining W q0 casts.
            for k in range(kt):
                if k < 2:
                    cast8(nc.vector,
                          qw8[:, k * i_core + 0:k * i_core + wsh_cols],
                          wf0[:, k * wsh_cols:(k + 1) * wsh_cols],
                          inv[:, 1:2])
                cast8(nc.vector, qx8[:, k * m_core:k * m_core + xsh_cols],
                      xsh[:, k * xsh_cols:(k + 1) * xsh_cols], inv[:, 0:1])
                if k >= 2:
                    cast8(nc.gpsimd,
                          qw8[:, k * i_core + 0:k * i_core + wsh_cols],
                          wf0[:, k * wsh_cols:(k + 1) * wsh_cols],
                          inv[:, 1:2])
            # layer 2: residual chains for x shard and W q0
            for k in range(kt):
                # x: t on ACT, d on DVE, rx8 on ACT
                t = work.tile([128, xsh_cols], F32, tag="txa", bufs=3,
                              name=f"txa_{k}")
                t_pass(nc.scalar, t[:],
                       xsh[:, k * xsh_cols:(k + 1) * xsh_cols], inv[:, 0:1])
                dd = work.tile([128, xsh_cols], F32, tag="dx", bufs=3)
                nc.vector.tensor_tensor(
                    dd[:], t[:], qx8[:, k * m_core:k * m_core + xsh_cols],
                    op=mybir.AluOpType.subtract)
                m_pass(nc.scalar, rx8[:, k * m_core:k * m_core + xsh_cols],
                       dd[:])
                # W q0: t on Pool, d on DVE, rw8 on Pool
                tw = work.tile([128, wsh_cols], F32, tag="tw", bufs=3,
                               name=f"tw0_{k}")
                t_pass(nc.gpsimd, tw[:],
                       wf0[:, k * wsh_cols:(k + 1) * wsh_cols], inv[:, 1:2])
                dw = work.tile([128, wsh_cols], F32, tag="dw", bufs=3)
                nc.vector.tensor_tensor(
                    dw[:], tw[:], qw8[:, k * i_core:k * i_core + wsh_cols],
                    op=mybir.AluOpType.subtract)
                m_pass(nc.gpsimd, rw8[:, k * i_core:k * i_core + wsh_cols],
                       dw[:])

            def quant_w_resid(k, q, src, t_eng, d_eng, r8_eng):
                c0 = q * wsh_cols
                tw = work.tile([128, wsh_cols], F32, tag="tw", bufs=3)
                t_pass(t_eng, tw[:], src, inv[:, 1:2])
                dw = work.tile([128, wsh_cols], F32, tag="dw", bufs=3)
                d_eng.tensor_tensor(
                    dw[:], tw[:],
                    qw8[:, k * i_core + c0:k * i_core + c0 + wsh_cols],
                    op=mybir.AluOpType.subtract)
                m_pass(r8_eng,
                       rw8[:, k * i_core + c0:k * i_core + c0 + wsh_cols],
                       dw[:])

            def quant_x_resid(k, src):
                c0 = xsh_cols
                t = work.tile([128, rw], F32, tag="txa", bufs=3)
                t_pass(nc.scalar, t[:], src, inv[:, 0:1])
                dd = work.tile([128, rw], F32, tag="dx", bufs=3)
                nc.vector.tensor_tensor(
                    dd[:], t[:],
                    qx8[:, k * m_core + c0:k * m_core + c0 + rw],
                    op=mybir.AluOpType.subtract)
                m_pass(nc.scalar,
                       rx8[:, k * m_core + c0:k * m_core + c0 + rw], dd[:])

            # W q1: direct casts (DVE) + residuals
            for k in range(kt):
                cast8(nc.vector,
                      qw8[:, k * i_core + wsh_cols:k * i_core + 2 * wsh_cols],
                      wrest[(1, k)][:], inv[:, 1:2])
            for k in range(kt):
                quant_w_resid(k, 1, wrest[(1, k)][:],
                              nc.vector if k % 2 else nc.gpsimd,
                              nc.vector,
                              nc.gpsimd if k % 2 else nc.vector)

            # ---- matmul + evac ----
            qxv = qx8[:, :].rearrange("p (k m) -> p k m", k=kt)
            rxv = rx8[:, :].rearrange("p (k m) -> p k m", k=kt)
            qwv = qw8[:, :].rearrange("p (k i) -> p k i", k=kt)
            rwv = rw8[:, :].rearrange("p (k i) -> p k i", k=kt)

            nq_it = wsh_cols // 128  # I-tiles per quad

            def mm_quad(q, tg0, sub=None):
                """One quad: 4 PSUM pairs. qw.qx parts kp-outer (PE consumes
                k-pairs as quant delivers), rw.qx next, then per-group qw.rx
                tails closing each group progressively so evacs overlap the
                remaining matmuls."""
                tiles = []
                for i in (range(nq_it) if sub is None else sub):
                    j = q * nq_it + i
                    ps = pp.tile([128, 1024], F32, tag="ps",
                                 name=f"ps_{j}_{tg0}")
                    tiles.append((j, ps))

                def mm(ps, ks, wslice, xv, c0, start, stop):
                    nc.tensor.matmul(
                        ps, wslice, xv[:, ks, c0:c0 + 512],
                        start=start, stop=stop, perf_mode=DR,
                    )

                for kp in range(kt // 2):
                    ks = slice(2 * kp, 2 * kp + 2)
                    for j, ps in tiles:
                        i0, i1 = j * 128, (j + 1) * 128
                        for half in range(2):
                            mm(ps[:, half * 512:(half + 1) * 512], ks,
                               qwv[:, ks, i0:i1], qxv, (tg0 + half) * 512,
                               kp == 0, False)
                for kp in range(kt // 2):
                    ks = slice(2 * kp, 2 * kp + 2)
                    for j, ps in tiles:
                        i0, i1 = j * 128, (j + 1) * 128
                        for half in range(2):
                            mm(ps[:, half * 512:(half + 1) * 512], ks,
                               rwv[:, ks, i0:i1], qxv, (tg0 + half) * 512,
                               False, False)
                for j, ps in tiles:
                    i0, i1 = j * 128, (j + 1) * 128
                    for kp in range(kt // 2):
                        ks = slice(2 * kp, 2 * kp + 2)
                        for half in range(2):
                            mm(ps[:, half * 512:(half + 1) * 512], ks,
                               qwv[:, ks, i0:i1], rxv, (tg0 + half) * 512,
                               False, kp == kt // 2 - 1)
                    ot = evac.tile([128, 1024], BF16, tag="ot")
                    nc.scalar.activation(
                        ot[:], ps[:], mybir.ActivationFunctionType.Gelu,
                        bias=bt[:, j:j + 1], scale=ss[:, 0:1],
                    )
                    nc.sync.dma_start(
                        outT[j * 128:(j + 1) * 128,
                             tg0 * 512:(tg0 + 2) * 512],
                        ot[:],
                    )

            # ---- sweep A (tg01); W q2/q3 casts+residuals and x tg23
            # quant spread across the quads below consumption rate ----
            for q in (2, 3):
                for k in range(kt):
                    cast8(nc.gpsimd if q == 2 else nc.vector,
                          qw8[:, k * i_core + q * wsh_cols:
                              k * i_core + (q + 1) * wsh_cols],
                          wrest[(q, k)][:], inv[:, 1:2])
            # ---- sweep A (tg01); W q2/q3 residuals + x tg23 quant
            # spread across the quads below consumption rate ----
            wsched = {0: [(2, k) for k in range(8)],
                      1: [(3, k) for k in range(6)],
                      2: [(3, 6), (3, 7)],
                      3: []}
            for q in range(n_wq):
                if q == 0:
                    mm_quad(q, 0, sub=[0, 1])
                    mm_quad(q, 0, sub=[2, 3])
                else:
                    mm_quad(q, 0)
                for (wq, k) in wsched[q]:
                    quant_w_resid(k, wq, wrest[(wq, k)][:],
                                  nc.gpsimd if k % 2 == 0 else nc.vector,
                                  nc.vector, nc.gpsimd)
                for k in range(q * 2, q * 2 + 2):
                    cast8(nc.vector,
                          qx8[:, k * m_core + xsh_cols:(k + 1) * m_core],
                          xrest[k][:], inv[:, 0:1])
                    quant_x_resid(k, xrest[k][:])
            # ---- sweep B (tg23) ----
            for q in range(n_wq):
                mm_quad(q, 2, sub=[0, 1])
                mm_quad(q, 2, sub=[2, 3])
    _split_sync_waits(nc)
    return nc


_CACHE: dict = {}


def _get_nc():
    if "nc" not in _CACHE:
        _CACHE["nc"] = build()
    return _CACHE["nc"]


def shard_inputs(x, W, b):
    """Host-side sharding: pure layout (transpose/slice/replicate), no math."""
    x2 = np.ascontiguousarray(x.reshape(M, H).T)  # [H, M]
    in_maps = []
    for c in range(N_CORES):
        ti, ii = c // II, c % II
        mq, ih = M // TI, I // II
        q = x2[:, ti * mq:(ti + 1) * mq]
        sh = mq // II
        perm = np.r_[ii * sh:(ii + 1) * sh, 0:ii * sh, (ii + 1) * sh:mq]
        xT = np.ascontiguousarray(q[:, perm])
        # permute W columns so this core's distinct 1/8 max-shard (an I-quad)
        # is block 0: its staging doubles as the shard max input
        wsd = ih // TI
        wperm = np.r_[ti * wsd:(ti + 1) * wsd, 0:ti * wsd, (ti + 1) * wsd:ih]
        wT = np.ascontiguousarray(W[ii * ih:(ii + 1) * ih, :].T[:, wperm])
        bia = np.ascontiguousarray(
            b[ii * ih:(ii + 1) * ih][wperm].reshape(ih // 128, 128).T
        )
        in_maps.append({"xT": xT, "wT": wT, "bias": bia})
    return in_maps


def unshard_output(results):
    """Assemble per-core transposed blocks into the full [B, S, I] output."""
    outT = np.empty((I, M), np.float32)
    for c in range(N_CORES):
        ti, ii = c // II, c % II
        mq, ih = M // TI, I // II
        sh = mq // II
        perm = np.r_[ii * sh:(ii + 1) * sh, 0:ii * sh, (ii + 1) * sh:mq]
        wsd = ih // TI
        wperm = np.r_[ti * wsd:(ti + 1) * wsd, 0:ti * wsd, (ti + 1) * wsd:ih]
        blk = np.asarray(results[c]["outT"]).astype(np.float32)
        outT[ii * ih:(ii + 1) * ih, ti * mq:(ti + 1) * mq] = \
            blk[np.argsort(wperm)][:, np.argsort(perm)]
    return np.ascontiguousarray(outT.T).reshape(B, S, I)


def kernel(x, W, b):
    nc = _get_nc()
    in_maps = shard_inputs(
        np.asarray(x, np.float32), np.asarray(W, np.float32),
        np.asarray(b, np.float32)
    )
    res = bass_utils.run_bass_kernel_spmd(nc, in_maps, core_ids=list(range(N_CORES)))
    return unshard_output(res.results)


# revision 62
# speedup vs baseline: 1.0007x; 1.0007x over previous
"""Trainium2 Bass kernel for nn_BertIntermediate (QuantizeLinear + exact GELU).

Reference computation:
    xq = fake_quant(x)   # symmetric per-tensor int8 fake quant, scale = max|x|/127
    Wq = fake_quant(W)
    h  = xq @ Wq.T + b
    out = h * 0.5 * (1 + erf(h/sqrt(2)))

Numerical scheme — e4m3 split + DoubleRow fp8 matmul:
  v = x/scale; the reference rounds q = rne(v), an integer in [-128, 127].
  Split q = q8 + r with q8 = e4m3(v) (a single tensor_scalar whose fp8 SBUF
  write performs the rounding) and r = rne(q - q8), computed exactly on
  device via the fp32 magic-number trick from t = v + 1.5*2^23 (which
  materializes rne(v)) and the actually-written q8. r is in [-4, 4] and
  exactly representable in fp8e4. Then
      qx.qw = qx8.qw8 + qx8.rw + rx.qw8 + rx.rw
  and the rx.rw term is dropped (~4e-4 of output absmax). The three kept
  groups run as fp8e4 DoubleRow matmuls (two contraction rows per PE pass,
  0.5 cycles/row): 6N cycles per [128, N] output tile vs bf16's 8N, i.e.
  PE time 82us vs 109us per core. Measured end-to-end rel err 8.2e-3
  (threshold 2e-2).

  The scales factor out of the GEMM into the scalar-engine activation:
  out = gelu(sx*sw*psum + bias), evacuated directly to bf16 (halves the
  output DMA; bf16 adds ~2e-3 rel max, still way in budget).

Sharding (8 cores): 2D grid, 4-way over tokens x 2-way over intermediate
(min DMA: ~10.5 MB in + 8.4 MB out per core). The global quantization
scales need max|x|, max|W| over the FULL tensors: each core reduces a
distinct 1/8 shard (host permutes token/I columns so its shard is block 0
of its inputs), a PE-transpose folds per-partition maxes, and one tiny
padded AllGather + local max yields the global scales on-device. The
collective has a fixed ~15us cost in the perf model and its input DMA
queues behind all previously-issued transfers, so every bulk DMA after the
shards is sync-gated on the cc ship to keep the queue clear.

Schedule: the AllGather window streams the remaining W/x chunks; after the
scales land, single-pass direct casts unblock the PE within ~3us while the
residual chains (t, d=t-q8, r8=d-magic) trail on ACT/DVE/Pool. Matmul
groups are emitted kp-outer across each I-quad's four PSUM pairs with the
rx-dependent third term per-group last, closing groups progressively so
gelu evacuations overlap the remaining matmuls.
"""

import numpy as np

import concourse.bass as bass
import concourse.mybir as mybir
from concourse import bass_utils
from concourse.tile import TileContext
from concourse.tile_rust import add_dep_helper

F32 = mybir.dt.float32
BF16 = mybir.dt.bfloat16
FP8 = mybir.dt.float8e4
MAGIC = 12582912.0  # 1.5 * 2**23: fp32 add/sub rounds to nearest int (RNE)
N_CORES = 8
TI, II = 4, 2  # token-quarters x intermediate-halves

# Full problem dims
B, S, H, I = 16, 512, 1024, 4096
M = B * S  # 8192 tokens

DR = mybir.MatmulPerfMode.DoubleRow


def _split_sync_waits(nc, max_waits=1):
    """Walrus in this container rejects instructions carrying more than a
    couple of sync-wait commands ("Too many sync wait commands"). Hoist excess
    waits onto single-wait nops inserted just before the instruction on the
    same engine queue — sequencers process in order, so semantics are
    unchanged."""
    n = 0
    for fn in nc.m.functions:
        for blk in fn.blocks:
            new_insts = []
            for inst in blk.instructions:
                si = inst.sync_info
                waits = list(si.on_wait or []) if si is not None else []
                if len(waits) > max_waits:
                    keep = waits[-max_waits:]
                    for w in waits[:-max_waits]:
                        n += 1
                        nop = mybir.InstNoOp(
                            name=f"I-waitsplit-{n}",
                            ins=[],
                            outs=[],
                            engine=inst.engine,
                        )
                        nop.sync_info = mybir.SyncInfo(on_wait=[w], on_update=[])
                        new_insts.append(nop)
                    inst.sync_info = mybir.SyncInfo(
                        on_wait=keep, on_update=list(si.on_update or [])
                    )
                new_insts.append(inst)
            blk.instructions = new_insts


def build(h=H, m_core=M // TI, i_core=I // II):
    """Build the SPMD Bass program for one core's block."""
    kt = h // 128          # contraction k-tiles
    n_it = i_core // 128   # output I-tiles
    n_tg = m_core // 512   # token groups
    xsh_cols = m_core // II   # this core's distinct x max-shard (cols 0..)
    wsh_cols = i_core // TI   # this core's distinct W max-shard (cols 0..)

    nc = bass.Bass(num_devices=N_CORES)
    xT = nc.dram_tensor("xT", [h, m_core], F32, kind="ExternalInput")
    wT = nc.dram_tensor("wT", [h, i_core], F32, kind="ExternalInput")
    bias = nc.dram_tensor("bias", [128, n_it], F32, kind="ExternalInput")
    outT = nc.dram_tensor("outT", [i_core, m_core], BF16, kind="ExternalOutput")
    CCW = 16  # payload padded to 64 B/rank; only the first 2 floats are used
    cc_w = nc.dram_tensor("cc_w", [1, CCW], F32, kind="Internal")
    cc_wo = nc.dram_tensor("cc_wo", [1, CCW * N_CORES], F32, kind="Internal",
                           addr_space="Shared")
    cc_x = nc.dram_tensor("cc_x", [1, CCW], F32, kind="Internal")
    cc_xo = nc.dram_tensor("cc_xo", [1, CCW * N_CORES], F32, kind="Internal",
                           addr_space="Shared")
    ident = nc.inline_tensor(np.eye(128, dtype=np.float32), name="ident128")

    groups = [list(range(N_CORES))]

    with TileContext(nc) as tc:
        with (
            tc.tile_pool(name="res", bufs=1) as res,
            tc.tile_pool(name="work", bufs=4) as work,
            tc.tile_pool(name="small", bufs=1) as small,
            tc.tile_pool(name="psum", bufs=4, space="PSUM") as pp,
            tc.tile_pool(name="evac", bufs=4) as evac,
        ):
            # persistent fp8 split tensors, layout [128, (k, cols)]
            qx8 = res.tile([128, kt * m_core], FP8, tag="qx8")
            rx8 = res.tile([128, kt * m_core], FP8, tag="rx8")
            qw8 = res.tile([128, kt * i_core], FP8, tag="qw8")
            rw8 = res.tile([128, kt * i_core], FP8, tag="rw8")
            # x max-shard staged in f32 until the scales arrive
            xsh = res.tile([128, kt * xsh_cols], F32, tag="xsh")
            wf0 = res.tile([128, kt * wsh_cols], F32, tag="wf0")

            macc = small.tile([128, 2 * kt + 2], F32, tag="macc")
            idt = small.tile([128, 128], F32, tag="idt")
            nc.sync.dma_start(idt[:], ident[:, :])

            # ---- phase 0: shard staging + maxes; W scales collective first
            # so W q0/q1 quantize during x's collective window ----
            for k in range(kt):
                nc.sync.dma_start(
                    wf0[:, k * wsh_cols:(k + 1) * wsh_cols],
                    wT[k * 128:(k + 1) * 128, 0:wsh_cols],
                )
                nc.vector.tensor_reduce(
                    macc[:, kt + 1 + k:kt + 2 + k],
                    wf0[:, k * wsh_cols:(k + 1) * wsh_cols],
                    axis=mybir.AxisListType.X,
                    op=mybir.AluOpType.max, apply_absolute_value=True,
                )
            for k in range(kt):
                if k < kt - 1:
                    parts = [(0, xsh_cols)]
                else:
                    parts = [(0, xsh_cols - 256), (xsh_cols - 256, xsh_cols)]
                for pi, (c0, c1) in enumerate(parts):
                    nc.sync.dma_start(
                        xsh[:, k * xsh_cols + c0:k * xsh_cols + c1],
                        xT[k * 128:(k + 1) * 128, c0:c1],
                    )
                    nc.vector.tensor_reduce(
                        macc[:, k:k + 1] if pi == 0 else macc[:, kt:kt + 1],
                        xsh[:, k * xsh_cols + c0:k * xsh_cols + c1],
                        axis=mybir.AxisListType.X,
                        op=mybir.AluOpType.max, apply_absolute_value=True,
                    )
            czero = small.tile([1, CCW], F32, tag="czero")
            nc.vector.memset(czero[:], 0.0)
            nc.sync.dma_start(cc_w[0:1, 2:CCW], czero[0:1, 2:CCW])
            nc.sync.dma_start(cc_x[0:1, 2:CCW], czero[0:1, 2:CCW])
            bt = small.tile([128, n_it], F32, tag="bt")
            nc.sync.dma_start(bt[:], bias[:, :])

            gm2 = small.tile([128, 2], F32, tag="gm2")
            lmax = small.tile([1, 2], F32, tag="lmax")
            sxsw = small.tile([128, 2], F32, tag="sxsw")
            inv = small.tile([128, 2], F32, tag="inv")
            g6t = small.tile([128, CCW * N_CORES], F32, tag="g6t")
            g6x = small.tile([128, CCW * N_CORES], F32, tag="g6x")
            gmxw = small.tile([128, CCW], F32, tag="gmxw")
            gmxx = small.tile([128, CCW], F32, tag="gmxx")
            n_wq = i_core // wsh_cols  # 4 I-quads

            # ---- quantize helpers ----
            def t_pass(t_eng, dst, src_ap, invc):
                if t_eng is nc.scalar:
                    nc.scalar.activation(
                        dst, src_ap, mybir.ActivationFunctionType.Copy,
                        bias=MAGIC, scale=invc,
                    )
                else:
                    t_eng.tensor_scalar(dst, src_ap, invc, MAGIC,
                                        op0=mybir.AluOpType.mult,
                                        op1=mybir.AluOpType.add)

            def m_pass(eng, dst, src_ap):
                """dst = src - MAGIC (fp8 write rounds to e4m3)."""
                if eng is nc.scalar:
                    nc.scalar.activation(
                        dst, src_ap, mybir.ActivationFunctionType.Copy,
                        bias=-MAGIC, scale=1.0,
                    )
                else:
                    eng.tensor_scalar(dst, src_ap, MAGIC, None,
                                      op0=mybir.AluOpType.subtract)

            # ---- scales: one AllGather carries both maxes ----
            for c in (1, 0):
                lo, hi = (kt + 1, 2 * kt + 1) if c == 1 else (0, kt + 1)
                nc.vector.tensor_reduce(
                    gm2[:, c:c + 1], macc[:, lo:hi], axis=mybir.AxisListType.X,
                    op=mybir.AluOpType.max,
                )
                gmt = pp.tile([1, 128], F32, tag="ps", name=f"gmt{c}")
                nc.tensor.transpose(gmt[:], gm2[:, c:c + 1], idt[:])
                nc.vector.tensor_reduce(
                    lmax[:, c:c + 1], gmt[:], axis=mybir.AxisListType.X,
                    op=mybir.AluOpType.max,
                )
            cc_dma = nc.sync.dma_start(cc_x[0:1, 0:2], lmax[0:1, 0:2])
            nc.gpsimd.collective_compute(
                "AllGather", mybir.AluOpType.bypass, replica_groups=groups,
                ins=[cc_x[:, :]], outs=[cc_xo[:, :]],
            )
            g6_dma = nc.sync.dma_start(
                g6x[:], cc_xo[0:1, :].broadcast_to([128, CCW * N_CORES])
            )
            nc.vector.tensor_reduce(
                gmxx[:], g6x[:, :].rearrange("p (r s) -> p s r", r=N_CORES),
                axis=mybir.AxisListType.X, op=mybir.AluOpType.max,
            )
            nc.vector.tensor_scalar_mul(sxsw[:], gmxx[:, 0:2], 1.0 / 127.0)
            nc.vector.reciprocal(inv[:], sxsw[:])
            ss = small.tile([128, 1], F32, tag="ss")
            nc.vector.tensor_tensor(
                ss[:], sxsw[:, 0:1], sxsw[:, 1:2], op=mybir.AluOpType.mult
            )

            # rest-of-input DMAs stream during/after the collective window;
            # gated on the cc ship so they cannot block it in the DMA FIFO.
            # All W quads first (their direct casts are PE gates), x rest after.
            wrest = {}
            for q in (1, 2, 3):
                for k in range(kt):
                    wf = work.tile([128, wsh_cols], F32, tag="wf", bufs=14,
                                   name=f"wf_{q}_{k}")
                    d = nc.sync.dma_start(
                        wf[:],
                        wT[k * 128:(k + 1) * 128,
                           q * wsh_cols:(q + 1) * wsh_cols],
                    )
                    add_dep_helper(d.ins, cc_dma.ins, sync=True,
                                   reason="keep DMA clear for cc ship")
                    wrest[(q, k)] = wf
            xrest = {}
            rw = m_core - xsh_cols
            for k in range(kt):
                xf = work.tile([128, rw], F32, tag="xf", bufs=4,
                               name=f"xrest_{k}")
                for h in range(2):
                    d = nc.sync.dma_start(
                        xf[:, h * 512:(h + 1) * 512],
                        xT[k * 128:(k + 1) * 128,
                           xsh_cols + h * 512:xsh_cols + (h + 1) * 512],
                    )
                    add_dep_helper(d.ins, cc_dma.ins, sync=True,
                                   reason="keep DMA clear for cc ship")
                xrest[k] = xf

            # ---- direct-cast front: qx8/qw8 are single tensor_scalar
            # casts (the fp8 write does the e4m3 rounding), so the PE's
            # qw.qx parts are gated only by these; the magic-rounded
            # residual chains (t, d, r8) trail behind ----
            def cast8(eng, dst, src_ap, invc):
                eng.tensor_scalar(dst, src_ap, invc, None,
                                  op0=mybir.AluOpType.mult)

            # layer 1: direct casts. DVE leads with the k0/k1 pairs the
            # first matmul needs; Pool covers the remaining W q0 casts.
            for k in range(kt):
                if k < 2:
                    cast8(nc.vector,
                          qw8[:, k * i_core + 0:k * i_core + wsh_cols],
                          wf0[:, k * wsh_cols:(k + 1) * wsh_cols],
                          inv[:, 1:2])
                cast8(nc.vector, qx8[:, k * m_core:k * m_core + xsh_cols],
                      xsh[:, k * xsh_cols:(k + 1) * xsh_cols], inv[:, 0:1])
                if k >= 2:
                    cast8(nc.gpsimd,
                          qw8[:, k * i_core + 0:k * i_core + wsh_cols],
                          wf0[:, k * wsh_cols:(k + 1) * wsh_cols],
                          inv[:, 1:2])
            # layer 2: residual chains for x shard and W q0
            for k in range(kt):
                # x: t on ACT, d on DVE, rx8 on ACT
                t = work.tile([128, xsh_cols], F32, tag="txa", bufs=3,
                              name=f"txa_{k}")
                t_pass(nc.scalar, t[:],
                       xsh[:, k * xsh_cols:(k + 1) * xsh_cols], inv[:, 0:1])
                dd = work.tile([128, xsh_cols], F32, tag="dx", bufs=3)
                nc.vector.tensor_tensor(
                    dd[:], t[:], qx8[:, k * m_core:k * m_core + xsh_cols],
                    op=mybir.AluOpType.subtract)
                m_pass(nc.scalar, rx8[:, k * m_core:k * m_core + xsh_cols],
                       dd[:])
                # W q0: t on Pool, d on DVE, rw8 on Pool
                tw = work.tile([128, wsh_cols], F32, tag="tw", bufs=3,
                               name=f"tw0_{k}")
                t_pass(nc.gpsimd, tw[:],
                       wf0[:, k * wsh_cols:(k + 1) * wsh_cols], inv[:, 1:2])
                dw = work.tile([128, wsh_cols], F32, tag="dw", bufs=3)
                nc.vector.tensor_tensor(
                    dw[:], tw[:], qw8[:, k * i_core:k * i_core + wsh_cols],
                    op=mybir.AluOpType.subtract)
                m_pass(nc.gpsimd, rw8[:, k * i_core:k * i_core + wsh_cols],
                       dw[:])

            def quant_w_resid(k, q, src, t_eng, d_eng, r8_eng):
                c0 = q * wsh_cols
                tw = work.tile([128, wsh_cols], F32, tag="tw", bufs=3)
                t_pass(t_eng, tw[:], src, inv[:, 1:2])
                dw = work.tile([128, wsh_cols], F32, tag="dw", bufs=3)
                d_eng.tensor_tensor(
                    dw[:], tw[:],
                    qw8[:, k * i_core + c0:k * i_core + c0 + wsh_cols],
                    op=mybir.AluOpType.subtract)
                m_pass(r8_eng,
                       rw8[:, k * i_core + c0:k * i_core + c0 + wsh_cols],
                       dw[:])

            def quant_x_resid(k, src):
                c0 = xsh_cols
                t = work.tile([128, rw], F32, tag="txa", bufs=3)
                t_pass(nc.scalar, t[:], src, inv[:, 0:1])
                dd = work.tile([128, rw], F32, tag="dx", bufs=3)
                nc.vector.tensor_tensor(
                    dd[:], t[:],
                    qx8[:, k * m_core + c0:k * m_core + c0 + rw],
                    op=mybir.AluOpType.subtract)
                m_pass(nc.scalar,
                       rx8[:, k * m_core + c0:k * m_core + c0 + rw], dd[:])

            # W q1: direct casts (DVE) + residuals
            for k in range(kt):
                cast8(nc.vector,
                      qw8[:, k * i_core + wsh_cols:k * i_core + 2 * wsh_cols],
                      wrest[(1, k)][:], inv[:, 1:2])
            for k in range(kt):
                quant_w_resid(k, 1, wrest[(1, k)][:],
                              nc.vector if k % 2 else nc.gpsimd,
                              nc.vector,
                              nc.gpsimd if k % 2 else nc.vector)

            # ---- matmul + evac ----
            qxv = qx8[:, :].rearrange("p (k m) -> p k m", k=kt)
            rxv = rx8[:, :].rearrange("p (k m) -> p k m", k=kt)
            qwv = qw8[:, :].rearrange("p (k i) -> p k i", k=kt)
            rwv = rw8[:, :].rearrange("p (k i) -> p k i", k=kt)

            nq_it = wsh_cols // 128  # I-tiles per quad

            def mm_quad(q, tg0, sub=None):
                """One quad: 4 PSUM pairs. qw.qx parts kp-outer (PE consumes
                k-pairs as quant delivers), rw.qx next, then per-group qw.rx
                tails closing each group progressively so evacs overlap the
                remaining matmuls."""
                tiles = []
                for i in (range(nq_it) if sub is None else sub):
                    j = q * nq_it + i
                    ps = pp.tile([128, 1024], F32, tag="ps",
                                 name=f"ps_{j}_{tg0}")
                    tiles.append((j, ps))

                def mm(ps, ks, wslice, xv, c0, start, stop):
                    nc.tensor.matmul(
                        ps, wslice, xv[:, ks, c0:c0 + 512],
                        start=start, stop=stop, perf_mode=DR,
                    )

                for kp in range(kt // 2):
                    ks = slice(2 * kp, 2 * kp + 2)
                    for j, ps in tiles:
                        i0, i1 = j * 128, (j + 1) * 128
                        for half in range(2):
                            mm(ps[:, half * 512:(half + 1) * 512], ks,
                               qwv[:, ks, i0:i1], qxv, (tg0 + half) * 512,
                               kp == 0, False)
                for kp in range(kt // 2):
                    ks = slice(2 * kp, 2 * kp + 2)
                    for j, ps in tiles:
                        i0, i1 = j * 128, (j + 1) * 128
                        for half in range(2):
                            mm(ps[:, half * 512:(half + 1) * 512], ks,
                               qwv[:, ks, i0:i1], rxv, (tg0 + half) * 512,
                               False, False)
                for j, ps in tiles:
                    i0, i1 = j * 128, (j + 1) * 128
                    for kp in range(kt // 2):
                        ks = slice(2 * kp, 2 * kp + 2)
                        for half in range(2):
                            mm(ps[:, half * 512:(half + 1) * 512], ks,
                               rwv[:, ks, i0:i1], qxv, (tg0 + half) * 512,
                               False, kp == kt // 2 - 1)
                    ot = evac.tile([128, 1024], BF16, tag="ot")
                    nc.scalar.activation(
                        ot[:], ps[:], mybir.ActivationFunctionType.Gelu,
                        bias=bt[:, j:j + 1], scale=ss[:, 0:1],
                    )
                    nc.sync.dma_start(
                        outT[j * 128:(j + 1) * 128,
                             tg0 * 512:(tg0 + 2) * 512],
                        ot[:],
                    )

            # ---- sweep A (tg01); W q2/q3 casts+residuals and x tg23
            # quant spread across the quads below consumption rate ----
            for q in (2, 3):
                for k in range(kt):
                    cast8(nc.gpsimd if q == 2 else nc.vector,
                          qw8[:, k * i_core + q * wsh_cols:
                              k * i_core + (q + 1) * wsh_cols],
                          wrest[(q, k)][:], inv[:, 1:2])
            # ---- sweep A (tg01); W q2/q3 residuals + x tg23 quant
            # spread across the quads below consumption rate ----
            wsched = {0: [(2, k) for k in range(8)],
                      1: [(3, k) for k in range(6)],
                      2: [(3, 6), (3, 7)],
                      3: []}
            for q in range(n_wq):
                if q == 0:
                    mm_quad(q, 0, sub=[0, 1])
                    mm_quad(q, 0, sub=[2, 3])
                else:
                    mm_quad(q, 0)
                for (wq, k) in wsched[q]:
                    quant_w_resid(k, wq, wrest[(wq, k)][:],
                                  nc.gpsimd if k % 2 == 0 else nc.vector,
                                  nc.vector, nc.gpsimd)
                for k in range(q * 2, q * 2 + 2):
                    cast8(nc.vector,
                          qx8[:, k * m_core + xsh_cols:(k + 1) * m_core],
                          xrest[k][:], inv[:, 0:1])
                    quant_x_resid(k, xrest[k][:])
            # ---- sweep B (tg23) ----
            for q in range(n_wq):
                mm_quad(q, 2, sub=[0, 1])
                mm_quad(q, 2, sub=[2, 3])
    _split_sync_waits(nc)
    return nc


_CACHE: dict = {}


def _get_nc():
    if "nc" not in _CACHE:
        _CACHE["nc"] = build()
    return _CACHE["nc"]


def shard_inputs(x, W, b):
    """Host-side sharding: pure layout (transpose/slice/replicate), no math."""
    x2 = np.ascontiguousarray(x.reshape(M, H).T)  # [H, M]
    in_maps = []
    for c in range(N_CORES):
        ti, ii = c // II, c % II
        mq, ih = M // TI, I // II
        q = x2[:, ti * mq:(ti + 1) * mq]
        sh = mq // II
        perm = np.r_[ii * sh:(ii + 1) * sh, 0:ii * sh, (ii + 1) * sh:mq]
        xT = np.ascontiguousarray(q[:, perm])
        # permute W columns so this core's distinct 1/8 max-shard (an I-quad)
        # is block 0: its staging doubles as the shard max input
        wsd = ih // TI
        wperm = np.r_[ti * wsd:(ti + 1) * wsd, 0:ti * wsd, (ti + 1) * wsd:ih]
        wT = np.ascontiguousarray(W[ii * ih:(ii + 1) * ih, :].T[:, wperm])
        bia = np.ascontiguousarray(
            b[ii * ih:(ii + 1) * ih][wperm].reshape(ih // 128, 128).T
        )
        in_maps.append({"xT": xT, "wT": wT, "bias": bia})
    return in_maps


def unshard_output(results):
    """Assemble per-core transposed blocks into the full [B, S, I] output."""
    outT = np.empty((I, M), np.float32)
    for c in range(N_CORES):
        ti, ii = c // II, c % II
        mq, ih = M // TI, I // II
        sh = mq // II
        perm = np.r_[ii * sh:(ii + 1) * sh, 0:ii * sh, (ii + 1) * sh:mq]
        wsd = ih // TI
        wperm = np.r_[ti * wsd:(ti + 1) * wsd, 0:ti * wsd, (ti + 1) * wsd:ih]
        blk = np.asarray(results[c]["outT"]).astype(np.float32)
        outT[ii * ih:(ii + 1) * ih, ti * mq:(ti + 1) * mq] = \
            blk[np.argsort(wperm)][:, np.argsort(perm)]
    return np.ascontiguousarray(outT.T).reshape(B, S, I)


def kernel(x, W, b):
    nc = _get_nc()
    in_maps = shard_inputs(
        np.asarray(x, np.float32), np.asarray(W, np.float32),
        np.asarray(b, np.float32)
    )
    res = bass_utils.run_bass_kernel_spmd(nc, in_maps, core_ids=list(range(N_CORES)))
    return unshard_output(res.results)


# revision 74
# speedup vs baseline: 1.0080x; 1.0073x over previous
"""Trainium2 Bass kernel for nn_BertIntermediate (QuantizeLinear + exact GELU).

Reference computation:
    xq = fake_quant(x)   # symmetric per-tensor int8 fake quant, scale = max|x|/127
    Wq = fake_quant(W)
    h  = xq @ Wq.T + b
    out = h * 0.5 * (1 + erf(h/sqrt(2)))

Numerical scheme — e4m3 split + DoubleRow fp8 matmul:
  v = x/scale; the reference rounds q = rne(v), an integer in [-128, 127].
  Split q = q8 + r with q8 = e4m3(v) (a single tensor_scalar whose fp8 SBUF
  write performs the rounding) and r = rne(q - q8), computed exactly on
  device via the fp32 magic-number trick from t = v + 1.5*2^23 (which
  materializes rne(v)) and the actually-written q8. r is in [-4, 4] and
  exactly representable in fp8e4. Then
      qx.qw = qx8.qw8 + qx8.rw + rx.qw8 + rx.rw
  and the rx.rw term is dropped (~4e-4 of output absmax). The three kept
  groups run as fp8e4 DoubleRow matmuls (two contraction rows per PE pass,
  0.5 cycles/row): 6N cycles per [128, N] output tile vs bf16's 8N, i.e.
  PE time 82us vs 109us per core. Measured end-to-end rel err 8.2e-3
  (threshold 2e-2).

  The scales factor out of the GEMM into the scalar-engine activation:
  out = gelu(sx*sw*psum + bias), evacuated directly to bf16 (halves the
  output DMA; bf16 adds ~2e-3 rel max, still way in budget).

Sharding (8 cores): 2D grid, 4-way over tokens x 2-way over intermediate
(min DMA: ~10.5 MB in + 8.4 MB out per core). The global quantization
scales need max|x|, max|W| over the FULL tensors: each core reduces a
distinct 1/8 shard (host permutes token/I columns so its shard is block 0
of its inputs), a PE-transpose folds per-partition maxes, and one tiny
padded AllGather + local max yields the global scales on-device. The
collective has a fixed ~15us cost in the perf model and its input DMA
queues behind all previously-issued transfers, so every bulk DMA after the
shards is sync-gated on the cc ship to keep the queue clear.

Schedule: the AllGather window streams the remaining W/x chunks; after the
scales land, single-pass direct casts unblock the PE within ~3us while the
residual chains (t, d=t-q8, r8=d-magic) trail on ACT/DVE/Pool. Matmul
groups are emitted kp-outer across each I-quad's four PSUM pairs with the
rx-dependent third term per-group last, closing groups progressively so
gelu evacuations overlap the remaining matmuls.
"""

import numpy as np

import concourse.bass as bass
import concourse.mybir as mybir
from concourse import bass_utils
from concourse.tile import TileContext
from concourse.tile_rust import add_dep_helper

F32 = mybir.dt.float32
BF16 = mybir.dt.bfloat16
FP8 = mybir.dt.float8e4
MAGIC = 12582912.0  # 1.5 * 2**23: fp32 add/sub rounds to nearest int (RNE)
N_CORES = 8
TI, II = 4, 2  # token-quarters x intermediate-halves

# Full problem dims
B, S, H, I = 16, 512, 1024, 4096
M = B * S  # 8192 tokens

DR = mybir.MatmulPerfMode.DoubleRow


def _split_sync_waits(nc, max_waits=1):
    """Walrus in this container rejects instructions carrying more than a
    couple of sync-wait commands ("Too many sync wait commands"). Hoist excess
    waits onto single-wait nops inserted just before the instruction on the
    same engine queue — sequencers process in order, so semantics are
    unchanged."""
    n = 0
    for fn in nc.m.functions:
        for blk in fn.blocks:
            new_insts = []
            for inst in blk.instructions:
                si = inst.sync_info
                waits = list(si.on_wait or []) if si is not None else []
                if len(waits) > max_waits:
                    keep = waits[-max_waits:]
                    for w in waits[:-max_waits]:
                        n += 1
                        nop = mybir.InstNoOp(
                            name=f"I-waitsplit-{n}",
                            ins=[],
                            outs=[],
                            engine=inst.engine,
                        )
                        nop.sync_info = mybir.SyncInfo(on_wait=[w], on_update=[])
                        new_insts.append(nop)
                    inst.sync_info = mybir.SyncInfo(
                        on_wait=keep, on_update=list(si.on_update or [])
                    )
                new_insts.append(inst)
            blk.instructions = new_insts


def build(h=H, m_core=M // TI, i_core=I // II):
    """Build the SPMD Bass program for one core's block."""
    kt = h // 128          # contraction k-tiles
    n_it = i_core // 128   # output I-tiles
    n_tg = m_core // 512   # token groups
    xsh_cols = m_core // II   # this core's distinct x max-shard (cols 0..)
    wsh_cols = i_core // TI   # this core's distinct W max-shard (cols 0..)

    nc = bass.Bass(num_devices=N_CORES)
    xT = nc.dram_tensor("xT", [h, m_core], F32, kind="ExternalInput")
    wT = nc.dram_tensor("wT", [h, i_core], F32, kind="ExternalInput")
    bias = nc.dram_tensor("bias", [128, n_it], F32, kind="ExternalInput")
    outT = nc.dram_tensor("outT", [i_core, m_core], BF16, kind="ExternalOutput")
    CCW = 16  # payload padded to 64 B/rank; only the first 2 floats are used
    cc_w = nc.dram_tensor("cc_w", [1, CCW], F32, kind="Internal")
    cc_wo = nc.dram_tensor("cc_wo", [1, CCW * N_CORES], F32, kind="Internal",
                           addr_space="Shared")
    cc_x = nc.dram_tensor("cc_x", [1, CCW], F32, kind="Internal")
    cc_xo = nc.dram_tensor("cc_xo", [1, CCW * N_CORES], F32, kind="Internal",
                           addr_space="Shared")
    ident = nc.inline_tensor(np.eye(128, dtype=np.float32), name="ident128")

    groups = [list(range(N_CORES))]

    with TileContext(nc) as tc:
        with (
            tc.tile_pool(name="res", bufs=1) as res,
            tc.tile_pool(name="work", bufs=4) as work,
            tc.tile_pool(name="small", bufs=1) as small,
            tc.tile_pool(name="psum", bufs=4, space="PSUM") as pp,
            tc.tile_pool(name="evac", bufs=3) as evac,
        ):
            # persistent fp8 split tensors, layout [128, (k, cols)]
            qx8 = res.tile([128, kt * m_core], FP8, tag="qx8")
            rx8 = res.tile([128, kt * m_core], FP8, tag="rx8")
            qw8 = res.tile([128, kt * i_core], FP8, tag="qw8")
            rw8 = res.tile([128, kt * i_core], FP8, tag="rw8")
            # x max-shard staged in f32 until the scales arrive
            xsh = res.tile([128, kt * xsh_cols], F32, tag="xsh")
            wf0 = res.tile([128, kt * wsh_cols], F32, tag="wf0")

            macc = small.tile([128, 2 * kt + 2], F32, tag="macc")
            idt = small.tile([128, 128], F32, tag="idt")
            nc.sync.dma_start(idt[:], ident[:, :])

            # ---- phase 0: shard staging + maxes; W scales collective first
            # so W q0/q1 quantize during x's collective window ----
            for k in range(kt):
                nc.sync.dma_start(
                    wf0[:, k * wsh_cols:(k + 1) * wsh_cols],
                    wT[k * 128:(k + 1) * 128, 0:wsh_cols],
                )
                nc.vector.tensor_reduce(
                    macc[:, kt + 1 + k:kt + 2 + k],
                    wf0[:, k * wsh_cols:(k + 1) * wsh_cols],
                    axis=mybir.AxisListType.X,
                    op=mybir.AluOpType.max, apply_absolute_value=True,
                )
            for k in range(kt):
                if k < kt - 1:
                    parts = [(0, xsh_cols)]
                else:
                    parts = [(0, xsh_cols - 256), (xsh_cols - 256, xsh_cols)]
                for pi, (c0, c1) in enumerate(parts):
                    nc.sync.dma_start(
                        xsh[:, k * xsh_cols + c0:k * xsh_cols + c1],
                        xT[k * 128:(k + 1) * 128, c0:c1],
                    )
                    nc.vector.tensor_reduce(
                        macc[:, k:k + 1] if pi == 0 else macc[:, kt:kt + 1],
                        xsh[:, k * xsh_cols + c0:k * xsh_cols + c1],
                        axis=mybir.AxisListType.X,
                        op=mybir.AluOpType.max, apply_absolute_value=True,
                    )
            czero = small.tile([1, CCW], F32, tag="czero")
            nc.vector.memset(czero[:], 0.0)
            nc.sync.dma_start(cc_w[0:1, 2:CCW], czero[0:1, 2:CCW])
            nc.sync.dma_start(cc_x[0:1, 2:CCW], czero[0:1, 2:CCW])
            bt = small.tile([128, n_it], F32, tag="bt")
            nc.sync.dma_start(bt[:], bias[:, :])

            gm2 = small.tile([128, 2], F32, tag="gm2")
            lmax = small.tile([1, 2], F32, tag="lmax")
            sxsw = small.tile([128, 2], F32, tag="sxsw")
            inv = small.tile([128, 2], F32, tag="inv")
            g6t = small.tile([128, CCW * N_CORES], F32, tag="g6t")
            g6x = small.tile([128, CCW * N_CORES], F32, tag="g6x")
            gmxw = small.tile([128, CCW], F32, tag="gmxw")
            gmxx = small.tile([128, CCW], F32, tag="gmxx")
            n_wq = i_core // wsh_cols  # 4 I-quads

            # ---- quantize helpers ----
            def t_pass(t_eng, dst, src_ap, invc):
                if t_eng is nc.scalar:
                    nc.scalar.activation(
                        dst, src_ap, mybir.ActivationFunctionType.Copy,
                        bias=MAGIC, scale=invc,
                    )
                else:
                    t_eng.tensor_scalar(dst, src_ap, invc, MAGIC,
                                        op0=mybir.AluOpType.mult,
                                        op1=mybir.AluOpType.add)

            def m_pass(eng, dst, src_ap):
                """dst = src - MAGIC (fp8 write rounds to e4m3)."""
                if eng is nc.scalar:
                    nc.scalar.activation(
                        dst, src_ap, mybir.ActivationFunctionType.Copy,
                        bias=-MAGIC, scale=1.0,
                    )
                else:
                    eng.tensor_scalar(dst, src_ap, MAGIC, None,
                                      op0=mybir.AluOpType.subtract)

            # ---- scales: one AllGather carries both maxes ----
            for c in (1, 0):
                lo, hi = (kt + 1, 2 * kt + 1) if c == 1 else (0, kt + 1)
                nc.vector.tensor_reduce(
                    gm2[:, c:c + 1], macc[:, lo:hi], axis=mybir.AxisListType.X,
                    op=mybir.AluOpType.max,
                )
                gmt = pp.tile([1, 128], F32, tag="ps", name=f"gmt{c}")
                nc.tensor.transpose(gmt[:], gm2[:, c:c + 1], idt[:])
                nc.vector.tensor_reduce(
                    lmax[:, c:c + 1], gmt[:], axis=mybir.AxisListType.X,
                    op=mybir.AluOpType.max,
                )
            cc_dma = nc.sync.dma_start(cc_x[0:1, 0:2], lmax[0:1, 0:2])
            nc.gpsimd.collective_compute(
                "AllGather", mybir.AluOpType.bypass, replica_groups=groups,
                ins=[cc_x[:, :]], outs=[cc_xo[:, :]],
            )
            g6_dma = nc.sync.dma_start(
                g6x[:], cc_xo[0:1, :].broadcast_to([128, CCW * N_CORES])
            )
            nc.vector.tensor_reduce(
                gmxx[:], g6x[:, :].rearrange("p (r s) -> p s r", r=N_CORES),
                axis=mybir.AxisListType.X, op=mybir.AluOpType.max,
            )
            nc.vector.tensor_scalar_mul(sxsw[:], gmxx[:, 0:2], 1.0 / 127.0)
            nc.vector.reciprocal(inv[:], sxsw[:])
            ss = small.tile([128, 1], F32, tag="ss")
            nc.vector.tensor_tensor(
                ss[:], sxsw[:, 0:1], sxsw[:, 1:2], op=mybir.AluOpType.mult
            )

            # rest-of-input DMAs stream during/after the collective window;
            # gated on the cc ship so they cannot block it in the DMA FIFO.
            # All W quads first (their direct casts are PE gates), x rest after.
            wrest = {}
            for q in (1, 2, 3):
                for k in range(kt):
                    wf = work.tile([128, wsh_cols], F32, tag="wf", bufs=12,
                                   name=f"wf_{q}_{k}")
                    d = nc.sync.dma_start(
                        wf[:],
                        wT[k * 128:(k + 1) * 128,
                           q * wsh_cols:(q + 1) * wsh_cols],
                    )
                    add_dep_helper(d.ins, cc_dma.ins, sync=True,
                                   reason="keep DMA clear for cc ship")
                    wrest[(q, k)] = wf
            xrest = {}
            rw = m_core - xsh_cols
            for k in range(kt):
                xf = work.tile([128, rw], F32, tag="xf", bufs=4,
                               name=f"xrest_{k}")
                for h in range(2):
                    d = nc.sync.dma_start(
                        xf[:, h * 512:(h + 1) * 512],
                        xT[k * 128:(k + 1) * 128,
                           xsh_cols + h * 512:xsh_cols + (h + 1) * 512],
                    )
                    add_dep_helper(d.ins, cc_dma.ins, sync=True,
                                   reason="keep DMA clear for cc ship")
                xrest[k] = xf

            # ---- direct-cast front: qx8/qw8 are single tensor_scalar
            # casts (the fp8 write does the e4m3 rounding), so the PE's
            # qw.qx parts are gated only by these; the magic-rounded
            # residual chains (t, d, r8) trail behind ----
            def cast8(eng, dst, src_ap, invc):
                eng.tensor_scalar(dst, src_ap, invc, None,
                                  op0=mybir.AluOpType.mult)

            # layer 1: direct casts. DVE leads with the k0/k1 pairs the
            # first matmul needs; x casts run at k-pair width after that;
            # Pool covers the remaining W q0 casts.
            vqx = qx8[:, :].rearrange("p (k m) -> p k m", k=kt)
            vrx = rx8[:, :].rearrange("p (k m) -> p k m", k=kt)
            for k in range(2):
                cast8(nc.vector,
                      qw8[:, k * i_core + 0:k * i_core + wsh_cols],
                      wf0[:, k * wsh_cols:(k + 1) * wsh_cols], inv[:, 1:2])
                cast8(nc.vector, qx8[:, k * m_core:k * m_core + xsh_cols],
                      xsh[:, k * xsh_cols:(k + 1) * xsh_cols], inv[:, 0:1])
            for kk in range(1, kt // 2):
                cast8(nc.vector, vqx[:, 2 * kk:2 * kk + 2, 0:xsh_cols],
                      xsh[:, 2 * kk * xsh_cols:(2 * kk + 2) * xsh_cols],
                      inv[:, 0:1])
                for k in (2 * kk, 2 * kk + 1):
                    cast8(nc.gpsimd,
                          qw8[:, k * i_core + 0:k * i_core + wsh_cols],
                          wf0[:, k * wsh_cols:(k + 1) * wsh_cols],
                          inv[:, 1:2])
            # layer 2: residual chains, x at k-pair width (contiguous in
            # xsh; strided pair view on the fp8 side), W q0 per-k
            for kk in range(kt // 2):
                t = work.tile([128, 2 * xsh_cols], F32, tag="txa", bufs=2,
                              name=f"txa_{kk}")
                t_pass(nc.scalar, t[:],
                       xsh[:, 2 * kk * xsh_cols:(2 * kk + 2) * xsh_cols],
                       inv[:, 0:1])
                dd = work.tile([128, 2 * xsh_cols], F32, tag="dx", bufs=2)
                nc.vector.tensor_tensor(
                    dd[:], t[:], vqx[:, 2 * kk:2 * kk + 2, 0:xsh_cols],
                    op=mybir.AluOpType.subtract)
                m_pass(nc.scalar, vrx[:, 2 * kk:2 * kk + 2, 0:xsh_cols],
                       dd[:])
                for k in (2 * kk, 2 * kk + 1):
                    tw = work.tile([128, wsh_cols], F32, tag="tw", bufs=3,
                                   name=f"tw0_{k}")
                    t_pass(nc.gpsimd, tw[:],
                           wf0[:, k * wsh_cols:(k + 1) * wsh_cols],
                           inv[:, 1:2])
                    dw = work.tile([128, wsh_cols], F32, tag="dw", bufs=3)
                    nc.vector.tensor_tensor(
                        dw[:], tw[:],
                        qw8[:, k * i_core:k * i_core + wsh_cols],
                        op=mybir.AluOpType.subtract)
                    m_pass(nc.gpsimd,
                           rw8[:, k * i_core:k * i_core + wsh_cols], dw[:])

            def quant_w_resid(k, q, src, t_eng, d_eng, r8_eng):
                c0 = q * wsh_cols
                tw = work.tile([128, wsh_cols], F32, tag="tw", bufs=3)
                t_pass(t_eng, tw[:], src, inv[:, 1:2])
                dw = work.tile([128, wsh_cols], F32, tag="dw", bufs=3)
                d_eng.tensor_tensor(
                    dw[:], tw[:],
                    qw8[:, k * i_core + c0:k * i_core + c0 + wsh_cols],
                    op=mybir.AluOpType.subtract)
                m_pass(r8_eng,
                       rw8[:, k * i_core + c0:k * i_core + c0 + wsh_cols],
                       dw[:])

            def quant_x_resid(k, src):
                c0 = xsh_cols
                t = work.tile([128, rw], F32, tag="txa", bufs=2)
                t_pass(nc.scalar, t[:], src, inv[:, 0:1])
                dd = work.tile([128, rw], F32, tag="dx", bufs=2)
                nc.vector.tensor_tensor(
                    dd[:], t[:],
                    qx8[:, k * m_core + c0:k * m_core + c0 + rw],
                    op=mybir.AluOpType.subtract)
                m_pass(nc.scalar,
                       rx8[:, k * m_core + c0:k * m_core + c0 + rw], dd[:])

            # W q1: direct casts (DVE) + residuals
            for k in range(kt):
                cast8(nc.vector,
                      qw8[:, k * i_core + wsh_cols:k * i_core + 2 * wsh_cols],
                      wrest[(1, k)][:], inv[:, 1:2])
            for k in range(kt):
                quant_w_resid(k, 1, wrest[(1, k)][:],
                              nc.vector if k % 2 else nc.gpsimd,
                              nc.vector,
                              nc.gpsimd if k % 2 else nc.vector)

            # ---- matmul + evac ----
            qxv = qx8[:, :].rearrange("p (k m) -> p k m", k=kt)
            rxv = rx8[:, :].rearrange("p (k m) -> p k m", k=kt)
            qwv = qw8[:, :].rearrange("p (k i) -> p k i", k=kt)
            rwv = rw8[:, :].rearrange("p (k i) -> p k i", k=kt)

            nq_it = wsh_cols // 128  # I-tiles per quad

            def mm_quad(q, tg0, sub=None):
                """One quad: 4 PSUM pairs. qw.qx parts kp-outer (PE consumes
                k-pairs as quant delivers), rw.qx next, then per-group qw.rx
                tails closing each group progressively so evacs overlap the
                remaining matmuls."""
                tiles = []
                for i in (range(nq_it) if sub is None else sub):
                    j = q * nq_it + i
                    ps = pp.tile([128, 1024], F32, tag="ps",
                                 name=f"ps_{j}_{tg0}")
                    tiles.append((j, ps))

                def mm(ps, ks, wslice, xv, c0, start, stop):
                    nc.tensor.matmul(
                        ps, wslice, xv[:, ks, c0:c0 + 512],
                        start=start, stop=stop, perf_mode=DR,
                    )

                for kp in range(kt // 2):
                    ks = slice(2 * kp, 2 * kp + 2)
                    for j, ps in tiles:
                        i0, i1 = j * 128, (j + 1) * 128
                        for half in range(2):
                            mm(ps[:, half * 512:(half + 1) * 512], ks,
                               qwv[:, ks, i0:i1], qxv, (tg0 + half) * 512,
                               kp == 0, False)
                for kp in range(kt // 2):
                    ks = slice(2 * kp, 2 * kp + 2)
                    for j, ps in tiles:
                        i0, i1 = j * 128, (j + 1) * 128
                        for half in range(2):
                            mm(ps[:, half * 512:(half + 1) * 512], ks,
                               qwv[:, ks, i0:i1], rxv, (tg0 + half) * 512,
                               False, False)
                for j, ps in tiles:
                    i0, i1 = j * 128, (j + 1) * 128
                    for kp in range(kt // 2):
                        ks = slice(2 * kp, 2 * kp + 2)
                        for half in range(2):
                            mm(ps[:, half * 512:(half + 1) * 512], ks,
                               rwv[:, ks, i0:i1], qxv, (tg0 + half) * 512,
                               False, kp == kt // 2 - 1)
                    ot = evac.tile([128, 1024], BF16, tag="ot")
                    nc.scalar.activation(
                        ot[:], ps[:], mybir.ActivationFunctionType.Gelu,
                        bias=bt[:, j:j + 1], scale=ss[:, 0:1],
                    )
                    nc.sync.dma_start(
                        outT[j * 128:(j + 1) * 128,
                             tg0 * 512:(tg0 + 2) * 512],
                        ot[:],
                    )

            # ---- sweep A (tg01); W q2/q3 casts+residuals and x tg23
            # quant spread across the quads below consumption rate ----
            for q in (2, 3):
                for k in range(kt):
                    cast8(nc.gpsimd if q == 2 else nc.vector,
                          qw8[:, k * i_core + q * wsh_cols:
                              k * i_core + (q + 1) * wsh_cols],
                          wrest[(q, k)][:], inv[:, 1:2])
            # ---- sweep A (tg01); W q2/q3 residuals + x tg23 quant
            # spread across the quads below consumption rate ----
            wsched = {0: [(2, k) for k in range(8)],
                      1: [(3, k) for k in range(6)],
                      2: [(3, 6), (3, 7)],
                      3: []}
            for q in range(n_wq):
                if q == 0:
                    mm_quad(q, 0, sub=[0, 1])
                    mm_quad(q, 0, sub=[2, 3])
                else:
                    mm_quad(q, 0)
                for (wq, k) in wsched[q]:
                    quant_w_resid(k, wq, wrest[(wq, k)][:],
                                  nc.gpsimd if k % 2 == 0 else nc.vector,
                                  nc.vector, nc.gpsimd)
                for k in range(q * 2, q * 2 + 2):
                    cast8(nc.vector,
                          qx8[:, k * m_core + xsh_cols:(k + 1) * m_core],
                          xrest[k][:], inv[:, 0:1])
                    quant_x_resid(k, xrest[k][:])
            # ---- sweep B (tg23) ----
            for q in range(n_wq):
                mm_quad(q, 2, sub=[0, 1])
                mm_quad(q, 2, sub=[2, 3])
    _split_sync_waits(nc)
    return nc


_CACHE: dict = {}


def _get_nc():
    if "nc" not in _CACHE:
        _CACHE["nc"] = build()
    return _CACHE["nc"]


def shard_inputs(x, W, b):
    """Host-side sharding: pure layout (transpose/slice/replicate), no math."""
    x2 = np.ascontiguousarray(x.reshape(M, H).T)  # [H, M]
    in_maps = []
    for c in range(N_CORES):
        ti, ii = c // II, c % II
        mq, ih = M // TI, I // II
        q = x2[:, ti * mq:(ti + 1) * mq]
        sh = mq // II
        perm = np.r_[ii * sh:(ii + 1) * sh, 0:ii * sh, (ii + 1) * sh:mq]
        xT = np.ascontiguousarray(q[:, perm])
        # permute W columns so this core's distinct 1/8 max-shard (an I-quad)
        # is block 0: its staging doubles as the shard max input
        wsd = ih // TI
        wperm = np.r_[ti * wsd:(ti + 1) * wsd, 0:ti * wsd, (ti + 1) * wsd:ih]
        wT = np.ascontiguousarray(W[ii * ih:(ii + 1) * ih, :].T[:, wperm])
        bia = np.ascontiguousarray(
            b[ii * ih:(ii + 1) * ih][wperm].reshape(ih // 128, 128).T
        )
        in_maps.append({"xT": xT, "wT": wT, "bias": bia})
    return in_maps


def unshard_output(results):
    """Assemble per-core transposed blocks into the full [B, S, I] output."""
    outT = np.empty((I, M), np.float32)
    for c in range(N_CORES):
        ti, ii = c // II, c % II
        mq, ih = M // TI, I // II
        sh = mq // II
        perm = np.r_[ii * sh:(ii + 1) * sh, 0:ii * sh, (ii + 1) * sh:mq]
        wsd = ih // TI
        wperm = np.r_[ti * wsd:(ti + 1) * wsd, 0:ti * wsd, (ti + 1) * wsd:ih]
        blk = np.asarray(results[c]["outT"]).astype(np.float32)
        outT[ii * ih:(ii + 1) * ih, ti * mq:(ti + 1) * mq] = \
            blk[np.argsort(wperm)][:, np.argsort(perm)]
    return np.ascontiguousarray(outT.T).reshape(B, S, I)


def kernel(x, W, b):
    nc = _get_nc()
    in_maps = shard_inputs(
        np.asarray(x, np.float32), np.asarray(W, np.float32),
        np.asarray(b, np.float32)
    )
    res = bass_utils.run_bass_kernel_spmd(nc, in_maps, core_ids=list(range(N_CORES)))
    return unshard_output(res.results)


# revision 76
# speedup vs baseline: 1.0099x; 1.0019x over previous
"""Trainium2 Bass kernel for nn_BertIntermediate (QuantizeLinear + exact GELU).

Reference computation:
    xq = fake_quant(x)   # symmetric per-tensor int8 fake quant, scale = max|x|/127
    Wq = fake_quant(W)
    h  = xq @ Wq.T + b
    out = h * 0.5 * (1 + erf(h/sqrt(2)))

Numerical scheme — e4m3 split + DoubleRow fp8 matmul:
  v = x/scale; the reference rounds q = rne(v), an integer in [-128, 127].
  Split q = q8 + r with q8 = e4m3(v) (a single tensor_scalar whose fp8 SBUF
  write performs the rounding) and r = rne(q - q8), computed exactly on
  device via the fp32 magic-number trick from t = v + 1.5*2^23 (which
  materializes rne(v)) and the actually-written q8. r is in [-4, 4] and
  exactly representable in fp8e4. Then
      qx.qw = qx8.qw8 + qx8.rw + rx.qw8 + rx.rw
  and the rx.rw term is dropped (~4e-4 of output absmax). The three kept
  groups run as fp8e4 DoubleRow matmuls (two contraction rows per PE pass,
  0.5 cycles/row): 6N cycles per [128, N] output tile vs bf16's 8N, i.e.
  PE time 82us vs 109us per core. Measured end-to-end rel err 8.2e-3
  (threshold 2e-2).

  The scales factor out of the GEMM into the scalar-engine activation:
  out = gelu(sx*sw*psum + bias), evacuated directly to bf16 (halves the
  output DMA; bf16 adds ~2e-3 rel max, still way in budget).

Sharding (8 cores): 2D grid, 4-way over tokens x 2-way over intermediate
(min DMA: ~10.5 MB in + 8.4 MB out per core). The global quantization
scales need max|x|, max|W| over the FULL tensors: each core reduces a
distinct 1/8 shard (host permutes token/I columns so its shard is block 0
of its inputs), a PE-transpose folds per-partition maxes, and one tiny
padded AllGather + local max yields the global scales on-device. The
collective has a fixed ~15us cost in the perf model and its input DMA
queues behind all previously-issued transfers, so every bulk DMA after the
shards is sync-gated on the cc ship to keep the queue clear.

Schedule: the AllGather window streams the remaining W/x chunks; after the
scales land, single-pass direct casts unblock the PE within ~3us while the
residual chains (t, d=t-q8, r8=d-magic) trail on ACT/DVE/Pool. Matmul
groups are emitted kp-outer across each I-quad's four PSUM pairs with the
rx-dependent third term per-group last, closing groups progressively so
gelu evacuations overlap the remaining matmuls.
"""

import numpy as np

import concourse.bass as bass
import concourse.mybir as mybir
from concourse import bass_utils
from concourse.tile import TileContext
from concourse.tile_rust import add_dep_helper

F32 = mybir.dt.float32
BF16 = mybir.dt.bfloat16
FP8 = mybir.dt.float8e4
MAGIC = 12582912.0  # 1.5 * 2**23: fp32 add/sub rounds to nearest int (RNE)
N_CORES = 8
TI, II = 4, 2  # token-quarters x intermediate-halves

# Full problem dims
B, S, H, I = 16, 512, 1024, 4096
M = B * S  # 8192 tokens

DR = mybir.MatmulPerfMode.DoubleRow


def _split_sync_waits(nc, max_waits=1):
    """Walrus in this container rejects instructions carrying more than a
    couple of sync-wait commands ("Too many sync wait commands"). Hoist excess
    waits onto single-wait nops inserted just before the instruction on the
    same engine queue — sequencers process in order, so semantics are
    unchanged."""
    n = 0
    for fn in nc.m.functions:
        for blk in fn.blocks:
            new_insts = []
            for inst in blk.instructions:
                si = inst.sync_info
                waits = list(si.on_wait or []) if si is not None else []
                if len(waits) > max_waits:
                    keep = waits[-max_waits:]
                    for w in waits[:-max_waits]:
                        n += 1
                        nop = mybir.InstNoOp(
                            name=f"I-waitsplit-{n}",
                            ins=[],
                            outs=[],
                            engine=inst.engine,
                        )
                        nop.sync_info = mybir.SyncInfo(on_wait=[w], on_update=[])
                        new_insts.append(nop)
                    inst.sync_info = mybir.SyncInfo(
                        on_wait=keep, on_update=list(si.on_update or [])
                    )
                new_insts.append(inst)
            blk.instructions = new_insts


def build(h=H, m_core=M // TI, i_core=I // II):
    """Build the SPMD Bass program for one core's block."""
    kt = h // 128          # contraction k-tiles
    n_it = i_core // 128   # output I-tiles
    n_tg = m_core // 512   # token groups
    xsh_cols = m_core // II   # this core's distinct x max-shard (cols 0..)
    wsh_cols = i_core // TI   # this core's distinct W max-shard (cols 0..)

    nc = bass.Bass(num_devices=N_CORES)
    xT = nc.dram_tensor("xT", [h, m_core], F32, kind="ExternalInput")
    wT = nc.dram_tensor("wT", [h, i_core], F32, kind="ExternalInput")
    bias = nc.dram_tensor("bias", [128, n_it], F32, kind="ExternalInput")
    outT = nc.dram_tensor("outT", [i_core, m_core], BF16, kind="ExternalOutput")
    CCW = 16  # payload padded to 64 B/rank; only the first 2 floats are used
    cc_w = nc.dram_tensor("cc_w", [1, CCW], F32, kind="Internal")
    cc_wo = nc.dram_tensor("cc_wo", [1, CCW * N_CORES], F32, kind="Internal",
                           addr_space="Shared")
    cc_x = nc.dram_tensor("cc_x", [1, CCW], F32, kind="Internal")
    cc_xo = nc.dram_tensor("cc_xo", [1, CCW * N_CORES], F32, kind="Internal",
                           addr_space="Shared")
    ident = nc.inline_tensor(np.eye(128, dtype=np.float32), name="ident128")

    groups = [list(range(N_CORES))]

    with TileContext(nc) as tc:
        with (
            tc.tile_pool(name="res", bufs=1) as res,
            tc.tile_pool(name="work", bufs=4) as work,
            tc.tile_pool(name="small", bufs=1) as small,
            tc.tile_pool(name="psum", bufs=4, space="PSUM") as pp,
            tc.tile_pool(name="evac", bufs=4) as evac,
        ):
            # persistent fp8 split tensors, layout [128, (k, cols)]
            qx8 = res.tile([128, kt * m_core], FP8, tag="qx8")
            rx8 = res.tile([128, kt * m_core], FP8, tag="rx8")
            qw8 = res.tile([128, kt * i_core], FP8, tag="qw8")
            rw8 = res.tile([128, kt * i_core], FP8, tag="rw8")
            # x max-shard staged in f32 until the scales arrive
            xsh = res.tile([128, kt * xsh_cols], F32, tag="xsh")
            wf0 = res.tile([128, kt * wsh_cols], F32, tag="wf0")

            macc = small.tile([128, 2 * kt + 2], F32, tag="macc")
            idt = small.tile([128, 128], F32, tag="idt")
            nc.sync.dma_start(idt[:], ident[:, :])

            # ---- phase 0: shard staging + maxes; W scales collective first
            # so W q0/q1 quantize during x's collective window ----
            for k in range(kt):
                nc.sync.dma_start(
                    wf0[:, k * wsh_cols:(k + 1) * wsh_cols],
                    wT[k * 128:(k + 1) * 128, 0:wsh_cols],
                )
                nc.vector.tensor_reduce(
                    macc[:, kt + 1 + k:kt + 2 + k],
                    wf0[:, k * wsh_cols:(k + 1) * wsh_cols],
                    axis=mybir.AxisListType.X,
                    op=mybir.AluOpType.max, apply_absolute_value=True,
                )
            for k in range(kt):
                if k < kt - 1:
                    parts = [(0, xsh_cols)]
                else:
                    parts = [(0, xsh_cols - 256), (xsh_cols - 256, xsh_cols)]
                for pi, (c0, c1) in enumerate(parts):
                    nc.sync.dma_start(
                        xsh[:, k * xsh_cols + c0:k * xsh_cols + c1],
                        xT[k * 128:(k + 1) * 128, c0:c1],
                    )
                    nc.vector.tensor_reduce(
                        macc[:, k:k + 1] if pi == 0 else macc[:, kt:kt + 1],
                        xsh[:, k * xsh_cols + c0:k * xsh_cols + c1],
                        axis=mybir.AxisListType.X,
                        op=mybir.AluOpType.max, apply_absolute_value=True,
                    )
            czero = small.tile([1, CCW], F32, tag="czero")
            nc.vector.memset(czero[:], 0.0)
            nc.sync.dma_start(cc_w[0:1, 2:CCW], czero[0:1, 2:CCW])
            nc.sync.dma_start(cc_x[0:1, 2:CCW], czero[0:1, 2:CCW])
            bt = small.tile([128, n_it], F32, tag="bt")
            nc.sync.dma_start(bt[:], bias[:, :])

            gm2 = small.tile([128, 2], F32, tag="gm2")
            lmax = small.tile([1, 2], F32, tag="lmax")
            sxsw = small.tile([128, 2], F32, tag="sxsw")
            inv = small.tile([128, 2], F32, tag="inv")
            g6t = small.tile([128, CCW * N_CORES], F32, tag="g6t")
            g6x = small.tile([128, CCW * N_CORES], F32, tag="g6x")
            gmxw = small.tile([128, CCW], F32, tag="gmxw")
            gmxx = small.tile([128, CCW], F32, tag="gmxx")
            n_wq = i_core // wsh_cols  # 4 I-quads

            # ---- quantize helpers ----
            def t_pass(t_eng, dst, src_ap, invc):
                if t_eng is nc.scalar:
                    nc.scalar.activation(
                        dst, src_ap, mybir.ActivationFunctionType.Copy,
                        bias=MAGIC, scale=invc,
                    )
                else:
                    t_eng.tensor_scalar(dst, src_ap, invc, MAGIC,
                                        op0=mybir.AluOpType.mult,
                                        op1=mybir.AluOpType.add)

            def m_pass(eng, dst, src_ap):
                """dst = src - MAGIC (fp8 write rounds to e4m3)."""
                if eng is nc.scalar:
                    nc.scalar.activation(
                        dst, src_ap, mybir.ActivationFunctionType.Copy,
                        bias=-MAGIC, scale=1.0,
                    )
                else:
                    eng.tensor_scalar(dst, src_ap, MAGIC, None,
                                      op0=mybir.AluOpType.subtract)

            # ---- scales: one AllGather carries both maxes ----
            for c in (1, 0):
                lo, hi = (kt + 1, 2 * kt + 1) if c == 1 else (0, kt + 1)
                nc.vector.tensor_reduce(
                    gm2[:, c:c + 1], macc[:, lo:hi], axis=mybir.AxisListType.X,
                    op=mybir.AluOpType.max,
                )
                gmt = pp.tile([1, 128], F32, tag="ps", name=f"gmt{c}")
                nc.tensor.transpose(gmt[:], gm2[:, c:c + 1], idt[:])
                nc.vector.tensor_reduce(
                    lmax[:, c:c + 1], gmt[:], axis=mybir.AxisListType.X,
                    op=mybir.AluOpType.max,
                )
            cc_dma = nc.sync.dma_start(cc_x[0:1, 0:2], lmax[0:1, 0:2])
            nc.gpsimd.collective_compute(
                "AllGather", mybir.AluOpType.bypass, replica_groups=groups,
                ins=[cc_x[:, :]], outs=[cc_xo[:, :]],
            )
            g6_dma = nc.sync.dma_start(
                g6x[:], cc_xo[0:1, :].broadcast_to([128, CCW * N_CORES])
            )
            nc.vector.tensor_reduce(
                gmxx[:], g6x[:, :].rearrange("p (r s) -> p s r", r=N_CORES),
                axis=mybir.AxisListType.X, op=mybir.AluOpType.max,
            )
            nc.vector.tensor_scalar_mul(sxsw[:], gmxx[:, 0:2], 1.0 / 127.0)
            nc.vector.reciprocal(inv[:], sxsw[:])
            ss = small.tile([128, 1], F32, tag="ss")
            nc.vector.tensor_tensor(
                ss[:], sxsw[:, 0:1], sxsw[:, 1:2], op=mybir.AluOpType.mult
            )

            # rest-of-input DMAs stream during/after the collective window;
            # gated on the cc ship so they cannot block it in the DMA FIFO.
            # All W quads first (their direct casts are PE gates), x rest after.
            wrest = {}
            for q in (1, 2, 3):
                for k in range(kt):
                    wf = work.tile([128, wsh_cols], F32, tag="wf", bufs=12,
                                   name=f"wf_{q}_{k}")
                    d = nc.sync.dma_start(
                        wf[:],
                        wT[k * 128:(k + 1) * 128,
                           q * wsh_cols:(q + 1) * wsh_cols],
                    )
                    add_dep_helper(d.ins, cc_dma.ins, sync=True,
                                   reason="keep DMA clear for cc ship")
                    wrest[(q, k)] = wf
            xrest = {}
            rw = m_core - xsh_cols
            for k in range(kt):
                xf = work.tile([128, rw], F32, tag="xf", bufs=3,
                               name=f"xrest_{k}")
                for h in range(2):
                    d = nc.sync.dma_start(
                        xf[:, h * 512:(h + 1) * 512],
                        xT[k * 128:(k + 1) * 128,
                           xsh_cols + h * 512:xsh_cols + (h + 1) * 512],
                    )
                    add_dep_helper(d.ins, cc_dma.ins, sync=True,
                                   reason="keep DMA clear for cc ship")
                xrest[k] = xf

            # ---- direct-cast front: qx8/qw8 are single tensor_scalar
            # casts (the fp8 write does the e4m3 rounding), so the PE's
            # qw.qx parts are gated only by these; the magic-rounded
            # residual chains (t, d, r8) trail behind ----
            def cast8(eng, dst, src_ap, invc):
                eng.tensor_scalar(dst, src_ap, invc, None,
                                  op0=mybir.AluOpType.mult)

            # layer 1: direct casts. DVE leads with the k0/k1 pairs the
            # first matmul needs; x casts run at k-pair width after that;
            # Pool covers the remaining W q0 casts.
            vqx = qx8[:, :].rearrange("p (k m) -> p k m", k=kt)
            vrx = rx8[:, :].rearrange("p (k m) -> p k m", k=kt)
            for k in range(2):
                cast8(nc.vector,
                      qw8[:, k * i_core + 0:k * i_core + wsh_cols],
                      wf0[:, k * wsh_cols:(k + 1) * wsh_cols], inv[:, 1:2])
                cast8(nc.vector, qx8[:, k * m_core:k * m_core + xsh_cols],
                      xsh[:, k * xsh_cols:(k + 1) * xsh_cols], inv[:, 0:1])
            for kk in range(1, kt // 2):
                cast8(nc.vector, vqx[:, 2 * kk:2 * kk + 2, 0:xsh_cols],
                      xsh[:, 2 * kk * xsh_cols:(2 * kk + 2) * xsh_cols],
                      inv[:, 0:1])
                for k in (2 * kk, 2 * kk + 1):
                    cast8(nc.gpsimd,
                          qw8[:, k * i_core + 0:k * i_core + wsh_cols],
                          wf0[:, k * wsh_cols:(k + 1) * wsh_cols],
                          inv[:, 1:2])
            # layer 2: residual chains, x at k-pair width (contiguous in
            # xsh; strided pair view on the fp8 side), W q0 per-k
            for kk in range(kt // 2):
                t = work.tile([128, 2 * xsh_cols], F32, tag="txa", bufs=2,
                              name=f"txa_{kk}")
                t_pass(nc.scalar, t[:],
                       xsh[:, 2 * kk * xsh_cols:(2 * kk + 2) * xsh_cols],
                       inv[:, 0:1])
                dd = work.tile([128, 2 * xsh_cols], F32, tag="dx", bufs=2)
                nc.vector.tensor_tensor(
                    dd[:], t[:], vqx[:, 2 * kk:2 * kk + 2, 0:xsh_cols],
                    op=mybir.AluOpType.subtract)
                m_pass(nc.scalar, vrx[:, 2 * kk:2 * kk + 2, 0:xsh_cols],
                       dd[:])
                for k in (2 * kk, 2 * kk + 1):
                    tw = work.tile([128, wsh_cols], F32, tag="tw", bufs=3,
                                   name=f"tw0_{k}")
                    t_pass(nc.gpsimd, tw[:],
                           wf0[:, k * wsh_cols:(k + 1) * wsh_cols],
                           inv[:, 1:2])
                    dw = work.tile([128, wsh_cols], F32, tag="dw", bufs=3)
                    nc.vector.tensor_tensor(
                        dw[:], tw[:],
                        qw8[:, k * i_core:k * i_core + wsh_cols],
                        op=mybir.AluOpType.subtract)
                    m_pass(nc.gpsimd,
                           rw8[:, k * i_core:k * i_core + wsh_cols], dw[:])

            def quant_w_resid(k, q, src, t_eng, d_eng, r8_eng):
                c0 = q * wsh_cols
                tw = work.tile([128, wsh_cols], F32, tag="tw", bufs=3)
                t_pass(t_eng, tw[:], src, inv[:, 1:2])
                dw = work.tile([128, wsh_cols], F32, tag="dw", bufs=3)
                d_eng.tensor_tensor(
                    dw[:], tw[:],
                    qw8[:, k * i_core + c0:k * i_core + c0 + wsh_cols],
                    op=mybir.AluOpType.subtract)
                m_pass(r8_eng,
                       rw8[:, k * i_core + c0:k * i_core + c0 + wsh_cols],
                       dw[:])

            def quant_x_resid(k, src):
                c0 = xsh_cols
                t = work.tile([128, rw], F32, tag="txa", bufs=2)
                t_pass(nc.scalar, t[:], src, inv[:, 0:1])
                dd = work.tile([128, rw], F32, tag="dx", bufs=2)
                nc.vector.tensor_tensor(
                    dd[:], t[:],
                    qx8[:, k * m_core + c0:k * m_core + c0 + rw],
                    op=mybir.AluOpType.subtract)
                m_pass(nc.scalar,
                       rx8[:, k * m_core + c0:k * m_core + c0 + rw], dd[:])

            # W q1: direct casts (DVE) + residuals
            for k in range(kt):
                cast8(nc.vector,
                      qw8[:, k * i_core + wsh_cols:k * i_core + 2 * wsh_cols],
                      wrest[(1, k)][:], inv[:, 1:2])
            for k in range(kt):
                quant_w_resid(k, 1, wrest[(1, k)][:],
                              nc.vector if k % 2 else nc.gpsimd,
                              nc.vector,
                              nc.gpsimd if k % 2 else nc.vector)

            # ---- matmul + evac ----
            qxv = qx8[:, :].rearrange("p (k m) -> p k m", k=kt)
            rxv = rx8[:, :].rearrange("p (k m) -> p k m", k=kt)
            qwv = qw8[:, :].rearrange("p (k i) -> p k i", k=kt)
            rwv = rw8[:, :].rearrange("p (k i) -> p k i", k=kt)

            nq_it = wsh_cols // 128  # I-tiles per quad

            def mm_quad(q, tg0, sub=None):
                """One quad: 4 PSUM pairs. qw.qx parts kp-outer (PE consumes
                k-pairs as quant delivers), rw.qx next, then per-group qw.rx
                tails closing each group progressively so evacs overlap the
                remaining matmuls."""
                tiles = []
                for i in (range(nq_it) if sub is None else sub):
                    j = q * nq_it + i
                    ps = pp.tile([128, 1024], F32, tag="ps",
                                 name=f"ps_{j}_{tg0}")
                    tiles.append((j, ps))

                def mm(ps, ks, wslice, xv, c0, start, stop):
                    nc.tensor.matmul(
                        ps, wslice, xv[:, ks, c0:c0 + 512],
                        start=start, stop=stop, perf_mode=DR,
                    )

                for kp in range(kt // 2):
                    ks = slice(2 * kp, 2 * kp + 2)
                    for j, ps in tiles:
                        i0, i1 = j * 128, (j + 1) * 128
                        for half in range(2):
                            mm(ps[:, half * 512:(half + 1) * 512], ks,
                               qwv[:, ks, i0:i1], qxv, (tg0 + half) * 512,
                               kp == 0, False)
                for kp in range(kt // 2):
                    ks = slice(2 * kp, 2 * kp + 2)
                    for j, ps in tiles:
                        i0, i1 = j * 128, (j + 1) * 128
                        for half in range(2):
                            mm(ps[:, half * 512:(half + 1) * 512], ks,
                               qwv[:, ks, i0:i1], rxv, (tg0 + half) * 512,
                               False, False)
                for j, ps in tiles:
                    i0, i1 = j * 128, (j + 1) * 128
                    for kp in range(kt // 2):
                        ks = slice(2 * kp, 2 * kp + 2)
                        for half in range(2):
                            mm(ps[:, half * 512:(half + 1) * 512], ks,
                               rwv[:, ks, i0:i1], qxv, (tg0 + half) * 512,
                               False, kp == kt // 2 - 1)
                    ot = evac.tile([128, 1024], BF16, tag="ot")
                    nc.scalar.activation(
                        ot[:], ps[:], mybir.ActivationFunctionType.Gelu,
                        bias=bt[:, j:j + 1], scale=ss[:, 0:1],
                    )
                    nc.sync.dma_start(
                        outT[j * 128:(j + 1) * 128,
                             tg0 * 512:(tg0 + 2) * 512],
                        ot[:],
                    )

            # ---- sweep A (tg01); W q2/q3 casts+residuals and x tg23
            # quant spread across the quads below consumption rate ----
            for q in (2, 3):
                for k in range(kt):
                    cast8(nc.gpsimd if q == 2 else nc.vector,
                          qw8[:, k * i_core + q * wsh_cols:
                              k * i_core + (q + 1) * wsh_cols],
                          wrest[(q, k)][:], inv[:, 1:2])
            # ---- sweep A (tg01); W q2/q3 residuals + x tg23 quant
            # spread across the quads below consumption rate ----
            wsched = {0: [(2, k) for k in range(8)],
                      1: [(3, k) for k in range(6)],
                      2: [(3, 6), (3, 7)],
                      3: []}
            for q in range(n_wq):
                if q == 0:
                    mm_quad(q, 0, sub=[0, 1])
                    mm_quad(q, 0, sub=[2, 3])
                else:
                    mm_quad(q, 0)
                for (wq, k) in wsched[q]:
                    quant_w_resid(k, wq, wrest[(wq, k)][:],
                                  nc.gpsimd if k % 2 == 0 else nc.vector,
                                  nc.vector, nc.gpsimd)
                for k in range(q * 2, q * 2 + 2):
                    cast8(nc.vector,
                          qx8[:, k * m_core + xsh_cols:(k + 1) * m_core],
                          xrest[k][:], inv[:, 0:1])
                    quant_x_resid(k, xrest[k][:])
            # ---- sweep B (tg23) ----
            for q in range(n_wq):
                mm_quad(q, 2, sub=[0, 1])
                mm_quad(q, 2, sub=[2, 3])
    _split_sync_waits(nc)
    return nc


_CACHE: dict = {}


def _get_nc():
    if "nc" not in _CACHE:
        _CACHE["nc"] = build()
    return _CACHE["nc"]


def shard_inputs(x, W, b):
    """Host-side sharding: pure layout (transpose/slice/replicate), no math."""
    x2 = np.ascontiguousarray(x.reshape(M, H).T)  # [H, M]
    in_maps = []
    for c in range(N_CORES):
        ti, ii = c // II, c % II
        mq, ih = M // TI, I // II
        q = x2[:, ti * mq:(ti + 1) * mq]
        sh = mq // II
        perm = np.r_[ii * sh:(ii + 1) * sh, 0:ii * sh, (ii + 1) * sh:mq]
        xT = np.ascontiguousarray(q[:, perm])
        # permute W columns so this core's distinct 1/8 max-shard (an I-quad)
        # is block 0: its staging doubles as the shard max input
        wsd = ih // TI
        wperm = np.r_[ti * wsd:(ti + 1) * wsd, 0:ti * wsd, (ti + 1) * wsd:ih]
        wT = np.ascontiguousarray(W[ii * ih:(ii + 1) * ih, :].T[:, wperm])
        bia = np.ascontiguousarray(
            b[ii * ih:(ii + 1) * ih][wperm].reshape(ih // 128, 128).T
        )
        in_maps.append({"xT": xT, "wT": wT, "bias": bia})
    return in_maps


def unshard_output(results):
    """Assemble per-core transposed blocks into the full [B, S, I] output."""
    outT = np.empty((I, M), np.float32)
    for c in range(N_CORES):
        ti, ii = c // II, c % II
        mq, ih = M // TI, I // II
        sh = mq // II
        perm = np.r_[ii * sh:(ii + 1) * sh, 0:ii * sh, (ii + 1) * sh:mq]
        wsd = ih // TI
        wperm = np.r_[ti * wsd:(ti + 1) * wsd, 0:ti * wsd, (ti + 1) * wsd:ih]
        blk = np.asarray(results[c]["outT"]).astype(np.float32)
        outT[ii * ih:(ii + 1) * ih, ti * mq:(ti + 1) * mq] = \
            blk[np.argsort(wperm)][:, np.argsort(perm)]
    return np.ascontiguousarray(outT.T).reshape(B, S, I)


def kernel(x, W, b):
    nc = _get_nc()
    in_maps = shard_inputs(
        np.asarray(x, np.float32), np.asarray(W, np.float32),
        np.asarray(b, np.float32)
    )
    res = bass_utils.run_bass_kernel_spmd(nc, in_maps, core_ids=list(range(N_CORES)))
    return unshard_output(res.results)
